# Initial kernel scaffold
#
"""Trainium2 Bass kernel for a 2-layer GRU LM step (T=64, B=64, E=512, H=1024, V=10000).

Strategy (8 NeuronCores, data-parallel over batch, 8 rows/core):
  - Layer-0 input transforms (x_emb @ W0^T + all layer-0 biases) precomputed in bulk
    on-device in transposed layout, staged to DRAM, injected per-step into PSUM via
    identity matmuls.
  - Sequential 64-step loop: both layers software-pipelined (layer 1 lags one step);
    layer-0 matmuls target PE column-group 0, layer-1 column-group 64 so their
    moving-operand streams overlap on the 128x128 array.
  - All matmul operands fp16 (1 cycle/column on TRN2 PE), fp32 PSUM accumulation.
  - Vocab projection (h1 @ Wout^T) done per 128-row block with fp16 weights streamed
    from DRAM.
"""

import os
import numpy as np

T, B, E, H, V = 64, 64, 512, 1024, 10000
NCORES = 8
BC = B // NCORES          # 8 batch rows per core
ROWS = T * BC             # 512 rows per core, row = t*BC + b
P = 128
HK = H // P               # 8
EK = E // P               # 4
G3 = 3 * H                # 3072
MC3 = G3 // P             # 24 column-chunks of the 3-gate cat
VCH = 500                 # vocab chunk (<=512 fp32 psum bank)
NV = V // VCH             # 20
WOUT_BLOCKS = ROWS // P   # 4 row blocks of 128

_CACHE = {}


def _f16(x):
    return np.ascontiguousarray(x.astype(np.float16))


def _f32(x):
    return np.ascontiguousarray(x.astype(np.float32))


def _build_module():
    """Build and compile the SPMD Bass module (same program on all 8 cores)."""
    import concourse.tile as tile
    from concourse import bacc, mybir

    f16, f32 = mybir.dt.float16, mybir.dt.float32

    nc = bacc.Bacc("TRN2", target_bir_lowering=False, debug=False,
                   num_devices=NCORES)

    # ---- DRAM I/O -------------------------------------------------------
    din = {}
    def dram_in(name, shape, dt):
        din[name] = nc.dram_tensor(name, shape, dt, kind="ExternalInput")
        return din[name]

    xembT_d = dram_in("xembT", [P, EK, ROWS], f16)      # per-core
    w0catT_d = dram_in("w0catT", [P, EK, G3], f16)
    b0_d = dram_in("b0", [P, MC3], f32)
    u0catT_d = dram_in("u0catT", [P, HK, G3], f16)
    w1catT_d = dram_in("w1catT", [P, HK, G3], f16)
    u1catT_d = dram_in("u1catT", [P, HK, G3], f16)
    b1rep_d = dram_in("b1rep", [P, MC3, BC], f16)
    i128_d = dram_in("i128", [P, P], f16)
    i8x_d = dram_in("i8x", [P, BC], f16)                # eye(8) at parts 0-7 and 64-71
    h0T_init_d = dram_in("h0T_init", [P, HK, BC], f16)  # per-core
    h1T_init_d = dram_in("h1T_init", [P, HK, BC], f16)  # per-core
    h0B_init_d = dram_in("h0B_init", [BC, H], f16)      # per-core
    h1B_init_d = dram_in("h1B_init", [BC, H], f16)      # per-core
    woutT_d = dram_in("woutT", [HK, NV, P, VCH], f16)

    logits_d = nc.dram_tensor("logits", [ROWS, V], f16, kind="ExternalOutput")
    hfin_d = nc.dram_tensor("hfin", [2, BC, H], f16, kind="ExternalOutput")

    with tile.TileContext(nc) as tc:
        _emit(tc, mybir, din, logits_d, hfin_d)

    nc.compile()
    return nc


def _emit(tc, mybir, din, logits_d, hfin_d):
    from contextlib import ExitStack

    nc = tc.nc
    f16, f32 = mybir.dt.float16, mybir.dt.float32
    Act = mybir.ActivationFunctionType
    Alu = mybir.AluOpType

    with ExitStack() as ctx:
        consts = ctx.enter_context(tc.tile_pool(name="consts", bufs=1))
        loopw = ctx.enter_context(tc.tile_pool(name="loopw", bufs=1))
        rxdram = ctx.enter_context(
            tc.tile_pool(name="rxdram", bufs=1, space="DRAM"))
        psmm = ctx.enter_context(
            tc.tile_pool(name="psmm", bufs=6, space="PSUM"))
        pstp = ctx.enter_context(
            tc.tile_pool(name="pstp", bufs=2, space="PSUM"))

        # ---- constants / loop weights (resident) ------------------------
        i128 = consts.tile([P, P], f16)
        nc.sync.dma_start(i128[:], din["i128"].ap())
        i8x = consts.tile([P, BC], f16)
        nc.sync.dma_start(i8x[:], din["i8x"].ap())
        b1rep = consts.tile([P, MC3, BC], f16)
        nc.sync.dma_start(b1rep[:], din["b1rep"].ap())
        b0 = consts.tile([P, MC3], f32)
        nc.sync.dma_start(b0[:], din["b0"].ap())

        u0catT = loopw.tile([P, HK, G3], f16)
        nc.sync.dma_start(u0catT[:], din["u0catT"].ap())
        w1catT = loopw.tile([P, HK, G3], f16)
        nc.sync.dma_start(w1catT[:], din["w1catT"].ap())
        u1catT = loopw.tile([P, HK, G3], f16)
        nc.sync.dma_start(u1catT[:], din["u1catT"].ap())

        rx0T_d = rxdram.tile([P, MC3, ROWS], f16)       # DRAM scratch

        # ---- bulk-0: rx0T[m, row] = W0cat @ x_emb^T + b0 (all L0 biases) --
        with tc.tile_pool(name="bulkw", bufs=1) as bulkw, \
             tc.tile_pool(name="bstg", bufs=3) as bstg:
            xembT = bulkw.tile([P, EK, ROWS], f16)
            nc.sync.dma_start(xembT[:], din["xembT"].ap())
            w0catT = bulkw.tile([P, EK, G3], f16)
            nc.sync.dma_start(w0catT[:], din["w0catT"].ap())
            for mc in range(MC3):
                ps = psmm.tile([P, 512], f32, tag="ps")
                for kc in range(EK):
                    nc.tensor.matmul(
                        ps[:, :ROWS], lhsT=w0catT[:, kc, mc * P:(mc + 1) * P],
                        rhs=xembT[:, kc, :], start=(kc == 0), stop=(kc == EK - 1),
                        tile_position=(0, 0), skip_group_check=True)
                stg = bstg.tile([P, ROWS], f16)
                nc.scalar.activation(stg[:], ps[:, :ROWS], Act.Identity,
                                     bias=b0[:, mc:mc + 1])
                nc.sync.dma_start(rx0T_d[:, mc, :], stg[:])

        # ---- state pools -------------------------------------------------
        hist = ctx.enter_context(tc.tile_pool(name="hist", bufs=1))
        state = ctx.enter_context(tc.tile_pool(name="state", bufs=2))
        elt = ctx.enter_context(tc.tile_pool(name="elt", bufs=2))
        rxp = ctx.enter_context(tc.tile_pool(name="rxp", bufs=3))
        wop = ctx.enter_context(tc.tile_pool(name="wop", bufs=3))
        lstg = ctx.enter_context(tc.tile_pool(name="lstg", bufs=3))

        h1Thist = hist.tile([P, HK, ROWS], f16)

        # initial state
        h0T = state.tile([P, HK, BC], f16, tag="h0T")
        nc.sync.dma_start(h0T[:], din["h0T_init"].ap())
        h1T = state.tile([P, HK, BC], f16, tag="h1T")
        nc.sync.dma_start(h1T[:], din["h1T_init"].ap())
        h0B = state.tile([BC, H], f16, tag="h0B")
        nc.sync.dma_start(h0B[:], din["h0B_init"].ap())
        h1B = state.tile([72, H], f16, tag="h1B")
        nc.sync.dma_start(h1B[64:72, :], din["h1B_init"].ap())

        MM = dict(skip_group_check=True)

        def gate_chain(lo, pos, rz_mms, ht_mms, rh_store_T=None):
            """Emit one layer-step: rz group -> sigmoid -> r*h -> transpose ->
            h~ group -> tanh -> h_new -> transpose.  lo: partition base (0 L0,
            64 L1).  rz_mms/ht_mms: callables emitting the accumulation MMs for
            one 512-column chunk j given (ps, j, first, last)."""
            pass  # structured inline below instead

        # --------------------------------------------------------------
        # main loop: tick t: L0 computes h0[t]; L1 computes h1[t-1]
        # --------------------------------------------------------------
        for t in range(T + 1):
            do_l0 = t < T
            do_l1 = t >= 1
            s = t - 1  # L1 step index

            if do_l0:
                rxt = rxp.tile([P, MC3, BC], f16)
                nc.sync.dma_start(rxt[:], rx0T_d[:, :, t * BC:(t + 1) * BC])

            # ---- L0: rz preacts (cols 0..2047 of [Ur0|Uz0]) ----
            if do_l0:
                rz0 = elt.tile([BC, 2 * H], f16, tag="rz0")
                for j in range(4):
                    ps = psmm.tile([P, 512], f32, tag="ps")
                    for kc in range(HK):
                        nc.tensor.matmul(
                            ps[0:BC, :], lhsT=h0T[:, kc, :],
                            rhs=u0catT[:, kc, j * 512:(j + 1) * 512],
                            start=(kc == 0), stop=False,
                            tile_position=(0, 0), **MM)
                    for ic in range(4):
                        mc = 4 * j + ic
                        nc.tensor.matmul(
                            ps[0:BC, ic * P:(ic + 1) * P], lhsT=rxt[:, mc, :],
                            rhs=i128[:], start=False, stop=(ic == 3),
                            tile_position=(0, 0), **MM)
                    nc.scalar.activation(rz0[0:BC, j * 512:(j + 1) * 512],
                                         ps[0:BC, :], Act.Sigmoid)

            if do_l1:
                rz1 = elt.tile([72, 2 * H], f16, tag="rz1")
                for j in range(4):
                    ps = psmm.tile([P, 512], f32, tag="ps")
                    for kc in range(HK):
                        nc.tensor.matmul(
                            ps[64:72, :], lhsT=h0T_prev[:, kc, :],
                            rhs=w1catT[:, kc, j * 512:(j + 1) * 512],
                            start=(kc == 0), stop=False,
                            tile_position=(0, 64), **MM)
                    for kc in range(HK):
                        nc.tensor.matmul(
                            ps[64:72, :], lhsT=h1T[:, kc, :],
                            rhs=u1catT[:, kc, j * 512:(j + 1) * 512],
                            start=False, stop=False,
                            tile_position=(0, 64), **MM)
                    for ic in range(4):
                        mc = 4 * j + ic
                        nc.tensor.matmul(
                            ps[64:72, ic * P:(ic + 1) * P], lhsT=b1rep[:, mc, :],
                            rhs=i128[:], start=False, stop=(ic == 3),
                            tile_position=(0, 64), **MM)
                    nc.scalar.activation(rz1[64:72, j * 512:(j + 1) * 512],
                                         ps[64:72, :], Act.Sigmoid)

            # ---- L0: r*h, transpose, h~ ----
            if do_l0:
                rh0 = elt.tile([BC, H], f16, tag="rh0")
                nc.vector.tensor_tensor(rh0[0:BC, :], rz0[0:BC, 0:H],
                                        h0B[0:BC, :], Alu.mult)
                pt = pstp.tile([P, HK, BC], f16, tag="pt")
                for kc in range(HK):
                    nc.tensor.matmul(pt[:, kc, :], lhsT=rh0[0:BC, kc * P:(kc + 1) * P],
                                     rhs=i8x[0:BC, :], is_transpose=True,
                                     start=(kc == 0), stop=(kc == HK - 1), **MM)
                rh0T = elt.tile([P, HK, BC], f16, tag="rh0T")
                nc.vector.tensor_copy(rh0T[:], pt[:])

                ht0 = elt.tile([BC, H], f16, tag="ht0")
                for j in range(2):
                    ps = psmm.tile([P, 512], f32, tag="ps")
                    for kc in range(HK):
                        nc.tensor.matmul(
                            ps[0:BC, :], lhsT=rh0T[:, kc, :],
                            rhs=u0catT[:, kc, 2048 + j * 512:2048 + (j + 1) * 512],
                            start=(kc == 0), stop=False,
                            tile_position=(0, 0), **MM)
                    for ic in range(4):
                        mc = 16 + 4 * j + ic
                        nc.tensor.matmul(
                            ps[0:BC, ic * P:(ic + 1) * P], lhsT=rxt[:, mc, :],
                            rhs=i128[:], start=False, stop=(ic == 3),
                            tile_position=(0, 0), **MM)
                    nc.scalar.activation(ht0[0:BC, j * 512:(j + 1) * 512],
                                         ps[0:BC, :], Act.Tanh)

                # h_new = h + z*(h~ - h)
                nc.gpsimd.tensor_tensor(ht0[0:BC, :], ht0[0:BC, :],
                                        h0B[0:BC, :], Alu.subtract)
                nc.vector.tensor_tensor(ht0[0:BC, :], ht0[0:BC, :],
                                        rz0[0:BC, H:2 * H], Alu.mult)
                h0B_new = state.tile([BC, H], f16, tag="h0B")
                nc.gpsimd.tensor_tensor(h0B_new[0:BC, :], ht0[0:BC, :],
                                        h0B[0:BC, :], Alu.add)
                pt2 = pstp.tile([P, HK, BC], f16, tag="pt")
                for kc in range(HK):
                    nc.tensor.matmul(pt2[:, kc, :],
                                     lhsT=h0B_new[0:BC, kc * P:(kc + 1) * P],
                                     rhs=i8x[0:BC, :], is_transpose=True,
                                     start=(kc == 0), stop=(kc == HK - 1), **MM)
                h0T_new = state.tile([P, HK, BC], f16, tag="h0T")
                nc.vector.tensor_copy(h0T_new[:], pt2[:])

            # ---- L1: r*h, transpose, h~ ----
            if do_l1:
                rh1 = elt.tile([72, H], f16, tag="rh1")
                nc.vector.tensor_tensor(rh1[64:72, :], rz1[64:72, 0:H],
                                        h1B[64:72, :], Alu.mult)
                pt = pstp.tile([P, HK, BC], f16, tag="pt")
                for kc in range(HK):
                    nc.tensor.matmul(pt[:, kc, :],
                                     lhsT=rh1[64:72, kc * P:(kc + 1) * P],
                                     rhs=i8x[64:72, :], is_transpose=True,
                                     start=(kc == 0), stop=(kc == HK - 1), **MM)
                rh1T = elt.tile([P, HK, BC], f16, tag="rh1T")
                nc.vector.tensor_copy(rh1T[:], pt[:])

                ht1 = elt.tile([72, H], f16, tag="ht1")
                for j in range(2):
                    ps = psmm.tile([P, 512], f32, tag="ps")
                    for kc in range(HK):
                        nc.tensor.matmul(
                            ps[64:72, :], lhsT=h0T_prev[:, kc, :],
                            rhs=w1catT[:, kc, 2048 + j * 512:2048 + (j + 1) * 512],
                            start=(kc == 0), stop=False,
                            tile_position=(0, 64), **MM)
                    for kc in range(HK):
                        nc.tensor.matmul(
                            ps[64:72, :], lhsT=rh1T[:, kc, :],
                            rhs=u1catT[:, kc, 2048 + j * 512:2048 + (j + 1) * 512],
                            start=False, stop=False,
                            tile_position=(0, 64), **MM)
                    for ic in range(4):
                        mc = 16 + 4 * j + ic
                        nc.tensor.matmul(
                            ps[64:72, ic * P:(ic + 1) * P], lhsT=b1rep[:, mc, :],
                            rhs=i128[:], start=False, stop=(ic == 3),
                            tile_position=(0, 64), **MM)
                    nc.scalar.activation(ht1[64:72, j * 512:(j + 1) * 512],
                                         ps[64:72, :], Act.Tanh)

                nc.gpsimd.tensor_tensor(ht1[64:72, :], ht1[64:72, :],
                                        h1B[64:72, :], Alu.subtract)
                nc.vector.tensor_tensor(ht1[64:72, :], ht1[64:72, :],
                                        rz1[64:72, H:2 * H], Alu.mult)
                h1B_new = state.tile([72, H], f16, tag="h1B")
                nc.gpsimd.tensor_tensor(h1B_new[64:72, :], ht1[64:72, :],
                                        h1B[64:72, :], Alu.add)
                pt2 = pstp.tile([P, HK, BC], f16, tag="pt")
                for kc in range(HK):
                    nc.tensor.matmul(pt2[:, kc, :],
                                     lhsT=h1B_new[64:72, kc * P:(kc + 1) * P],
                                     rhs=i8x[64:72, :], is_transpose=True,
                                     start=(kc == 0), stop=(kc == HK - 1), **MM)
                h1T_new = state.tile([P, HK, BC], f16, tag="h1T")
                nc.vector.tensor_copy(h1T_new[:], pt2[:])
                # also append into the history used by the vocab projection
                nc.vector.tensor_copy(h1Thist[:, :, s * BC:(s + 1) * BC], pt2[:])

            # rotate state
            if do_l0:
                h0T_prev = h0T
                h0T = h0T_new
                h0B = h0B_new
            if do_l1:
                h1T = h1T_new
                h1B = h1B_new

        # final hidden state out (fp16)
        hf0 = lstg.tile([BC, H], f16, tag="hf")
        nc.vector.tensor_copy(hf0[:], h0B[0:BC, :])
        nc.sync.dma_start(hfin_d.ap()[0], hf0[:])
        hf1 = lstg.tile([72, H], f16, tag="hf1")
        nc.vector.tensor_copy(hf1[64:72, :], h1B[64:72, :])
        nc.sync.dma_start(hfin_d.ap()[1], hf1[64:72, :])

        # ---- vocab projection: logits[rows, v] = h1 @ Wout^T ------------
        for m in range(WOUT_BLOCKS):
            for vc in range(NV):
                ps = psmm.tile([P, 512], f32, tag="ps")
                for kc in range(HK):
                    wt = wop.tile([P, VCH], f16, tag="wt")
                    nc.sync.dma_start(wt[:], din["woutT"].ap()[kc, vc])
                    nc.tensor.matmul(
                        ps[:, :VCH], lhsT=h1Thist[:, kc, m * P:(m + 1) * P],
                        rhs=wt[:], start=(kc == 0), stop=(kc == HK - 1),
                        tile_position=(0, 0), **MM)
                ls = lstg.tile([P, VCH], f16, tag="ls")
                nc.scalar.copy(ls[:], ps[:, :VCH])
                nc.sync.dma_start(
                    logits_d.ap()[m * P:(m + 1) * P, vc * VCH:(vc + 1) * VCH],
                    ls[:])


def _prep_host(inputs):
    """Pack host-side arrays: shared weights + per-core tensors."""
    W = {k: np.asarray(v) for k, v in inputs.items()}
    x_emb = W["emb"][W["inputs"]]                      # [T, B, E] fp32

    W0cat = np.concatenate([W["Wr0"], W["Wz0"], W["Wh0"]], 0)      # [3H, E]
    b0cat = np.concatenate([W["bur"][0], W["buz"][0], W["buh"][0]])
    U0cat = np.concatenate([W["Ur"][0], W["Uz"][0], W["Uh"][0]], 0)  # [3H, H]
    W1cat = np.concatenate([W["WrR"][0], W["WzR"][0], W["WhR"][0]], 0)
    b1cat = np.concatenate([W["brR"][0] + W["bur"][1],
                            W["bzR"][0] + W["buz"][1],
                            W["bhR"][0] + W["buh"][1]])
    U1cat = np.concatenate([W["Ur"][1], W["Uz"][1], W["Uh"][1]], 0)

    i8x = np.zeros((P, BC), np.float16)
    i8x[0:BC, :] = np.eye(BC, dtype=np.float16)
    i8x[64:64 + BC, :] = np.eye(BC, dtype=np.float16)

    shared = {
        "w0catT": _f16(W0cat.reshape(G3, EK, P).transpose(2, 1, 0)),
        "b0": _f32(b0cat.reshape(MC3, P).T),
        "u0catT": _f16(U0cat.reshape(G3, HK, P).transpose(2, 1, 0)),
        "w1catT": _f16(W1cat.reshape(G3, HK, P).transpose(2, 1, 0)),
        "u1catT": _f16(U1cat.reshape(G3, HK, P).transpose(2, 1, 0)),
        "b1rep": _f16(np.repeat(b1cat.reshape(MC3, P).T[:, :, None], BC, 2)),
        "i128": np.eye(P, dtype=np.float16),
        "i8x": i8x,
        "woutT": _f16(W["Wout"].reshape(NV, VCH, HK, P).transpose(2, 0, 3, 1)),
    }

    in_maps = []
    for c in range(NCORES):
        xe = x_emb[:, c * BC:(c + 1) * BC, :].reshape(ROWS, E)
        h0 = W["hidden"][0, c * BC:(c + 1) * BC, :]     # [BC, H]
        h1 = W["hidden"][1, c * BC:(c + 1) * BC, :]
        m = dict(shared)
        m["xembT"] = _f16(xe.reshape(ROWS, EK, P).transpose(2, 1, 0))
        m["h0T_init"] = _f16(h0.reshape(BC, HK, P).transpose(2, 1, 0))
        m["h1T_init"] = _f16(h1.reshape(BC, HK, P).transpose(2, 1, 0))
        m["h0B_init"] = _f16(h0)
        m["h1B_init"] = _f16(h1)
        in_maps.append(m)
    return in_maps


def kernel(**inputs):
    from concourse.bass_utils import run_bass_kernel_spmd

    if "nc" not in _CACHE:
        _CACHE["nc"] = _build_module()
    nc = _CACHE["nc"]

    in_maps = _prep_host(inputs)
    res = run_bass_kernel_spmd(nc, in_maps, core_ids=list(range(NCORES)))

    logits = np.zeros((T, B, V), np.float32)
    hfin = np.zeros((2, B, H), np.float32)
    for c in range(NCORES):
        lg = res.results[c]["logits"].astype(np.float32)   # [ROWS, V]
        logits[:, c * BC:(c + 1) * BC, :] = lg.reshape(T, BC, V)
        hfin[:, c * BC:(c + 1) * BC, :] = res.results[c]["hfin"].astype(np.float32)

    logits += np.asarray(inputs["bout"], np.float32)[None, None, :]
    return logits, hfin


# revision 4
# speedup vs baseline: 1.1134x; 1.1134x over previous
"""Trainium2 Bass kernel for a 2-layer GRU LM step (T=64, B=64, E=512, H=1024, V=10000).

Strategy (8 NeuronCores, data-parallel over batch, 8 rows/core):
  - Layer-0 input transforms (x_emb @ W0^T + all layer-0 biases) precomputed in bulk
    on-device in transposed layout, staged to DRAM, injected per-step into PSUM via
    identity matmuls.
  - Sequential 64-step loop, both layers software-pipelined (layer 1 lags one step),
    instruction emission interleaved between the two chains so the tensor engine
    stays fed while either chain is in its sigmoid/eltwise tail.
  - All matmul operands fp16 (1 cycle/column on the TRN2 PE), fp32 PSUM accumulation,
    fp16 elementwise state.
  - Vocab projection (h1 @ Wout^T) interleaved into the loop every 16 steps as
    tensor-engine filler work; fp16 weights streamed from DRAM.
"""

import numpy as np

T, B, E, H, V = 64, 64, 512, 1024, 10000
NCORES = 8
BC = B // NCORES          # 8 batch rows per core
ROWS = T * BC             # 512 rows per core, row = t*BC + b
P = 128
HK = H // P               # 8
EK = E // P               # 4
G3 = 3 * H                # 3072
MC3 = G3 // P             # 24 column-chunks of the 3-gate cat
VCH = 500                 # vocab chunk (<=512 fp32 psum bank)
NV = V // VCH             # 20
WOUT_BLOCKS = ROWS // P   # 4 row blocks of 128

_CACHE = {}


def _f16(x):
    return np.ascontiguousarray(x.astype(np.float16))


def _f32(x):
    return np.ascontiguousarray(x.astype(np.float32))


def _build_module(nrep=1):
    import concourse.tile as tile
    from concourse import bacc, mybir

    f16, f32 = mybir.dt.float16, mybir.dt.float32

    nc = bacc.Bacc("TRN2", target_bir_lowering=False, debug=False,
                   num_devices=NCORES)

    din = {}
    def dram_in(name, shape, dt):
        din[name] = nc.dram_tensor(name, shape, dt, kind="ExternalInput")
        return din[name]

    dram_in("xembT", [P, EK, ROWS], f16)       # per-core
    dram_in("w0catT", [P, EK, G3], f16)
    dram_in("b0", [P, MC3], f32)
    dram_in("u0catT", [P, HK, G3], f16)
    dram_in("w1catT", [P, HK, G3], f16)
    dram_in("u1catT", [P, HK, G3], f16)
    dram_in("b1B", [BC, G3], f16)              # L1 biases, replicated over batch
    dram_in("i128", [P, P], f16)
    dram_in("i8", [BC, BC], f16)
    dram_in("h0T_init", [P, HK, BC], f16)      # per-core
    dram_in("h1T_init", [P, HK, BC], f16)      # per-core
    dram_in("h0B_init", [BC, H], f16)          # per-core
    dram_in("h1B_init", [BC, H], f16)          # per-core
    dram_in("woutT", [HK, NV, P, VCH], f16)

    logits_d = nc.dram_tensor("logits", [ROWS, V], f16, kind="ExternalOutput")
    hfin_d = nc.dram_tensor("hfin", [2, BC, H], f16, kind="ExternalOutput")

    with tile.TileContext(nc) as tc:
        _emit(tc, mybir, din, logits_d, hfin_d, nrep)

    nc.compile()
    return nc


def _emit(tc, mybir, din, logits_d, hfin_d, nrep=1):
    from contextlib import ExitStack

    nc = tc.nc
    f16, f32 = mybir.dt.float16, mybir.dt.float32
    Act = mybir.ActivationFunctionType
    Alu = mybir.AluOpType
    MM = dict(skip_group_check=True, tile_position=(0, 0))

    with ExitStack() as ctx:
        consts = ctx.enter_context(tc.tile_pool(name="consts", bufs=1))
        loopw = ctx.enter_context(tc.tile_pool(name="loopw", bufs=1))
        rxdram = ctx.enter_context(
            tc.tile_pool(name="rxdram", bufs=1, space="DRAM"))
        psmm = ctx.enter_context(
            tc.tile_pool(name="psmm", bufs=6, space="PSUM"))
        pstp = ctx.enter_context(
            tc.tile_pool(name="pstp", bufs=2, space="PSUM"))
        hist = ctx.enter_context(tc.tile_pool(name="hist", bufs=1))
        state = ctx.enter_context(tc.tile_pool(name="state", bufs=2))
        elt = ctx.enter_context(tc.tile_pool(name="elt", bufs=1))
        rxp = ctx.enter_context(tc.tile_pool(name="rxp", bufs=2))
        wop = ctx.enter_context(tc.tile_pool(name="wop", bufs=4))
        lstg = ctx.enter_context(tc.tile_pool(name="lstg", bufs=2))

        for _rep in range(nrep):
            # ---- constants / loop weights (resident) ---------------------
            i128 = consts.tile([P, P], f16)
            nc.sync.dma_start(i128[:], din["i128"].ap())
            i8 = consts.tile([BC, BC], f16)
            nc.sync.dma_start(i8[:], din["i8"].ap())
            b1B = consts.tile([BC, G3], f16)
            nc.sync.dma_start(b1B[:], din["b1B"].ap())
            b0 = consts.tile([P, MC3], f32)
            nc.sync.dma_start(b0[:], din["b0"].ap())

            u0catT = loopw.tile([P, HK, G3], f16)
            nc.sync.dma_start(u0catT[:], din["u0catT"].ap())
            w1catT = loopw.tile([P, HK, G3], f16)
            nc.sync.dma_start(w1catT[:], din["w1catT"].ap())
            u1catT = loopw.tile([P, HK, G3], f16)
            nc.sync.dma_start(u1catT[:], din["u1catT"].ap())

            rx0T_d = rxdram.tile([P, MC3, ROWS], f16)

            # ---- bulk-0: rx0T = W0cat @ x_emb^T + b0 ---------------------
            with tc.tile_pool(name="bulkw", bufs=1) as bulkw, \
                 tc.tile_pool(name="bw0", bufs=6) as bw0, \
                 tc.tile_pool(name="bstg", bufs=3) as bstg:
                xembT = bulkw.tile([P, EK, ROWS], f16)
                nc.sync.dma_start(xembT[:], din["xembT"].ap())
                for mc in range(MC3):
                    ps = psmm.tile([P, 512], f32, tag="ps")
                    for kc in range(EK):
                        w0t = bw0.tile([P, P], f16, tag="w0t")
                        nc.sync.dma_start(
                            w0t[:], din["w0catT"].ap()[:, kc, mc * P:(mc + 1) * P])
                        nc.tensor.matmul(
                            ps[:, :ROWS], lhsT=w0t[:], rhs=xembT[:, kc, :],
                            start=(kc == 0), stop=(kc == EK - 1), **MM)
                    stg = bstg.tile([P, ROWS], f16)
                    nc.scalar.activation(stg[:], ps[:, :ROWS], Act.Identity,
                                         bias=b0[:, mc:mc + 1])
                    nc.sync.dma_start(rx0T_d[:, mc, :], stg[:])

            # initial state
            h0T = state.tile([P, HK, BC], f16, tag="h0T")
            nc.sync.dma_start(h0T[:], din["h0T_init"].ap())
            h1T = state.tile([P, HK, BC], f16, tag="h1T")
            nc.sync.dma_start(h1T[:], din["h1T_init"].ap())
            h0B = state.tile([BC, H], f16, tag="h0B")
            nc.sync.dma_start(h0B[:], din["h0B_init"].ap())
            h1B = state.tile([BC, H], f16, tag="h1B")
            nc.sync.dma_start(h1B[:], din["h1B_init"].ap())

            h1Thist = hist.tile([P, HK, ROWS], f16)

            def rz_chunk_l0(j, rxt, h0T):
                ps = psmm.tile([P, 512], f32, tag="ps")
                for ic in range(4):
                    mc = 4 * j + ic
                    nc.tensor.matmul(ps[0:BC, ic * P:(ic + 1) * P],
                                     lhsT=rxt[:, mc, :], rhs=i128[:],
                                     start=(ic == 0), stop=False, **MM)
                for kc in range(HK):
                    nc.tensor.matmul(ps[0:BC, :], lhsT=h0T[:, kc, :],
                                     rhs=u0catT[:, kc, j * 512:(j + 1) * 512],
                                     start=False, stop=(kc == HK - 1), **MM)
                return ps

            def rz_chunk_l1(j, h0T, h1T):
                ps = psmm.tile([P, 512], f32, tag="ps")
                for kc in range(HK):
                    nc.tensor.matmul(ps[0:BC, :], lhsT=h0T[:, kc, :],
                                     rhs=w1catT[:, kc, j * 512:(j + 1) * 512],
                                     start=(kc == 0), stop=False, **MM)
                for kc in range(HK):
                    nc.tensor.matmul(ps[0:BC, :], lhsT=h1T[:, kc, :],
                                     rhs=u1catT[:, kc, j * 512:(j + 1) * 512],
                                     start=False, stop=(kc == HK - 1), **MM)
                return ps

            def ht_chunk_l0(j, rxt, rh0T):
                ps = psmm.tile([P, 512], f32, tag="ps")
                for ic in range(4):
                    mc = 16 + 4 * j + ic
                    nc.tensor.matmul(ps[0:BC, ic * P:(ic + 1) * P],
                                     lhsT=rxt[:, mc, :], rhs=i128[:],
                                     start=(ic == 0), stop=False, **MM)
                for kc in range(HK):
                    nc.tensor.matmul(
                        ps[0:BC, :], lhsT=rh0T[:, kc, :],
                        rhs=u0catT[:, kc, 2048 + j * 512:2048 + (j + 1) * 512],
                        start=False, stop=(kc == HK - 1), **MM)
                return ps

            def ht_chunk_l1(j, h0T, rh1T):
                ps = psmm.tile([P, 512], f32, tag="ps")
                for kc in range(HK):
                    nc.tensor.matmul(
                        ps[0:BC, :], lhsT=h0T[:, kc, :],
                        rhs=w1catT[:, kc, 2048 + j * 512:2048 + (j + 1) * 512],
                        start=(kc == 0), stop=False, **MM)
                for kc in range(HK):
                    nc.tensor.matmul(
                        ps[0:BC, :], lhsT=rh1T[:, kc, :],
                        rhs=u1catT[:, kc, 2048 + j * 512:2048 + (j + 1) * 512],
                        start=False, stop=(kc == HK - 1), **MM)
                return ps

            def transpose8(src, tag):
                """src [BC, H] fp16 -> fp16 tile [P, HK, BC] (the transpose)."""
                pt = pstp.tile([P, HK, BC], f16, tag="pt")
                for kc in range(HK):
                    nc.tensor.matmul(pt[:, kc, :],
                                     lhsT=src[0:BC, kc * P:(kc + 1) * P],
                                     rhs=i8[:], is_transpose=True,
                                     start=(kc == 0), stop=(kc == HK - 1),
                                     skip_group_check=True)
                dst = elt.tile([P, HK, BC], f16, tag=tag)
                nc.vector.tensor_copy(dst[:], pt[:])
                return dst, pt

            def wout_group(m, vc):
                ps = psmm.tile([P, 512], f32, tag="ps")
                for kc in range(HK):
                    wt = wop.tile([P, VCH], f16, tag="wt")
                    nc.sync.dma_start(wt[:], din["woutT"].ap()[kc, vc])
                    nc.tensor.matmul(ps[:, :VCH],
                                     lhsT=h1Thist[:, kc, m * P:(m + 1) * P],
                                     rhs=wt[:], start=(kc == 0),
                                     stop=(kc == HK - 1), **MM)
                ls = lstg.tile([P, VCH], f16, tag="ls")
                nc.scalar.copy(ls[:], ps[:, :VCH])
                nc.sync.dma_start(
                    logits_d.ap()[m * P:(m + 1) * P, vc * VCH:(vc + 1) * VCH],
                    ls[:])

            # ----------------------------------------------------------
            # main loop: tick t: L0 computes h0[t]; L1 computes h1[t-1];
            # every 16 ticks a finished 128-row block of h1 history flows
            # through the vocab projection as PE filler.
            # ----------------------------------------------------------
            for t in range(T + 1):
                do_l0 = t < T
                do_l1 = t >= 1
                s = t - 1

                if do_l0:
                    rxt = rxp.tile([P, MC3, BC], f16)
                    nc.sync.dma_start(rxt[:], rx0T_d[:, :, t * BC:(t + 1) * BC])
                    rz0 = elt.tile([BC, 2 * H], f16, tag="rz0")
                if do_l1:
                    rz1 = elt.tile([BC, 2 * H], f16, tag="rz1")

                # interleaved rz matmul groups + per-chunk sigmoid
                for j in range(4):
                    if do_l0:
                        ps = rz_chunk_l0(j, rxt, h0T)
                        nc.scalar.activation(rz0[0:BC, j * 512:(j + 1) * 512],
                                             ps[0:BC, :], Act.Sigmoid)
                    if do_l1:
                        ps = rz_chunk_l1(j, h0T, h1T)
                        nc.vector.tensor_tensor(ps[0:BC, :], ps[0:BC, :],
                                                b1B[:, j * 512:(j + 1) * 512],
                                                Alu.add)
                        nc.scalar.activation(rz1[0:BC, j * 512:(j + 1) * 512],
                                             ps[0:BC, :], Act.Sigmoid)

                # r*h and transposes
                if do_l0:
                    rh0 = elt.tile([BC, H], f16, tag="rh0")
                    nc.vector.tensor_tensor(rh0[:], rz0[0:BC, 0:H], h0B[:],
                                            Alu.mult)
                    rh0T, _ = transpose8(rh0, "rh0T")
                if do_l1:
                    rh1 = elt.tile([BC, H], f16, tag="rh1")
                    nc.vector.tensor_tensor(rh1[:], rz1[0:BC, 0:H], h1B[:],
                                            Alu.mult)
                    rh1T, _ = transpose8(rh1, "rh1T")

                # h~ matmul groups + tanh
                if do_l0:
                    ht0 = elt.tile([BC, H], f16, tag="ht0")
                if do_l1:
                    ht1 = elt.tile([BC, H], f16, tag="ht1")
                for j in range(2):
                    if do_l0:
                        ps = ht_chunk_l0(j, rxt, rh0T)
                        nc.scalar.activation(ht0[0:BC, j * 512:(j + 1) * 512],
                                             ps[0:BC, :], Act.Tanh)
                    if do_l1:
                        ps = ht_chunk_l1(j, h0T, rh1T)
                        nc.vector.tensor_tensor(
                            ps[0:BC, :], ps[0:BC, :],
                            b1B[:, 2048 + j * 512:2048 + (j + 1) * 512], Alu.add)
                        nc.scalar.activation(ht1[0:BC, j * 512:(j + 1) * 512],
                                             ps[0:BC, :], Act.Tanh)

                # h_new = h + z*(h~ - h); transpose for next-step stationaries
                if do_l0:
                    nc.vector.tensor_tensor(ht0[:], ht0[:], h0B[:], Alu.subtract)
                    nc.vector.tensor_tensor(ht0[:], ht0[:], rz0[0:BC, H:2 * H],
                                            Alu.mult)
                    h0B_new = state.tile([BC, H], f16, tag="h0B")
                    nc.vector.tensor_tensor(h0B_new[:], ht0[:], h0B[:], Alu.add)
                    h0T_new, _ = transpose8(h0B_new, "h0Tn")
                if do_l1:
                    nc.vector.tensor_tensor(ht1[:], ht1[:], h1B[:], Alu.subtract)
                    nc.vector.tensor_tensor(ht1[:], ht1[:], rz1[0:BC, H:2 * H],
                                            Alu.mult)
                    h1B_new = state.tile([BC, H], f16, tag="h1B")
                    nc.vector.tensor_tensor(h1B_new[:], ht1[:], h1B[:], Alu.add)
                    pt = pstp.tile([P, HK, BC], f16, tag="pt")
                    for kc in range(HK):
                        nc.tensor.matmul(pt[:, kc, :],
                                         lhsT=h1B_new[0:BC, kc * P:(kc + 1) * P],
                                         rhs=i8[:], is_transpose=True,
                                         start=(kc == 0), stop=(kc == HK - 1),
                                         skip_group_check=True)
                    h1T_new = state.tile([P, HK, BC], f16, tag="h1T")
                    nc.vector.tensor_copy(h1T_new[:], pt[:])
                    nc.vector.tensor_copy(h1Thist[:, :, s * BC:(s + 1) * BC],
                                          pt[:])

                # rotate python refs
                if do_l0:
                    h0T = h0T_new
                    h0B = h0B_new
                if do_l1:
                    h1T = h1T_new
                    h1B = h1B_new

                # vocab projection filler: block m ready after tick 16(m+1)
                if t % 16 == 0 and t >= 16:
                    m = t // 16 - 1
                    for vc in range(NV):
                        wout_group(m, vc)

            # final hidden state out (fp16)
            nc.sync.dma_start(hfin_d.ap()[0], h0B[:])
            nc.sync.dma_start(hfin_d.ap()[1], h1B[:])


def _prep_host(inputs):
    W = {k: np.asarray(v) for k, v in inputs.items()}
    x_emb = W["emb"][W["inputs"]]                      # [T, B, E] fp32

    W0cat = np.concatenate([W["Wr0"], W["Wz0"], W["Wh0"]], 0)      # [3H, E]
    b0cat = np.concatenate([W["bur"][0], W["buz"][0], W["buh"][0]])
    U0cat = np.concatenate([W["Ur"][0], W["Uz"][0], W["Uh"][0]], 0)
    W1cat = np.concatenate([W["WrR"][0], W["WzR"][0], W["WhR"][0]], 0)
    b1cat = np.concatenate([W["brR"][0] + W["bur"][1],
                            W["bzR"][0] + W["buz"][1],
                            W["bhR"][0] + W["buh"][1]])
    U1cat = np.concatenate([W["Ur"][1], W["Uz"][1], W["Uh"][1]], 0)

    shared = {
        "w0catT": _f16(W0cat.reshape(G3, EK, P).transpose(2, 1, 0)),
        "b0": _f32(b0cat.reshape(MC3, P).T),
        "u0catT": _f16(U0cat.reshape(G3, HK, P).transpose(2, 1, 0)),
        "w1catT": _f16(W1cat.reshape(G3, HK, P).transpose(2, 1, 0)),
        "u1catT": _f16(U1cat.reshape(G3, HK, P).transpose(2, 1, 0)),
        "b1B": _f16(np.broadcast_to(b1cat[None, :], (BC, G3))),
        "i128": np.eye(P, dtype=np.float16),
        "i8": np.eye(BC, dtype=np.float16),
        "woutT": _f16(W["Wout"].reshape(NV, VCH, HK, P).transpose(2, 0, 3, 1)),
    }

    in_maps = []
    for c in range(NCORES):
        xe = x_emb[:, c * BC:(c + 1) * BC, :].reshape(ROWS, E)
        h0 = W["hidden"][0, c * BC:(c + 1) * BC, :]
        h1 = W["hidden"][1, c * BC:(c + 1) * BC, :]
        m = dict(shared)
        m["xembT"] = _f16(xe.reshape(ROWS, EK, P).transpose(2, 1, 0))
        m["h0T_init"] = _f16(h0.reshape(BC, HK, P).transpose(2, 1, 0))
        m["h1T_init"] = _f16(h1.reshape(BC, HK, P).transpose(2, 1, 0))
        m["h0B_init"] = _f16(h0)
        m["h1B_init"] = _f16(h1)
        in_maps.append(m)
    return in_maps


def kernel(**inputs):
    from concourse.bass_utils import run_bass_kernel_spmd

    if "nc" not in _CACHE:
        _CACHE["nc"] = _build_module()
    nc = _CACHE["nc"]

    in_maps = _prep_host(inputs)
    res = run_bass_kernel_spmd(nc, in_maps, core_ids=list(range(NCORES)))

    logits = np.zeros((T, B, V), np.float32)
    hfin = np.zeros((2, B, H), np.float32)
    for c in range(NCORES):
        lg = res.results[c]["logits"].astype(np.float32)
        logits[:, c * BC:(c + 1) * BC, :] = lg.reshape(T, BC, V)
        hfin[:, c * BC:(c + 1) * BC, :] = res.results[c]["hfin"].astype(np.float32)

    logits += np.asarray(inputs["bout"], np.float32)[None, None, :]
    return logits, hfin
